# revision 23
# baseline (speedup 1.0000x reference)
"""Bass/Tile kernel for bidirectional multi-head self-attention on 8 trn2 cores.

Problem: x[4, 2048, 1024], W_qkv[3072, 1024], W_proj[1024, 1024], H=16 heads,
Dh=64.  out = proj(softmax(q k^T / sqrt(Dh)) v).

Sharding: core c = (batch b = c//2, head-group g = c%2).  Each core computes
attention for 8 heads of one batch and a full-T partial output projection
(contraction over its 512 C_in columns); host sums the pair partials and
stacks batches.  x is transposed on the host so no PE transposes are needed.

Per-core pipeline (all matmuls bf16 in / fp32 psum accumulate):
  phase 1: qT/kT [dh,T] from stationary-W matmuls over xT; v [T,dh] tiles.
  phase 2: per head-quad, software-pipelined over kt:
           - scores row-packed: 2 heads concurrently on array row-halves
             (K=64 contraction each) into one [128,1024] psum pair tile
           - exp split between ScalarE (native Exp) and DVE (Schraudolph
             int16-bitcast fast exp; its constant bias cancels in softmax)
           - AV col-packed: 2 heads concurrently on array col-halves
           - denominators via ones[128,64] lhsT matmuls: row sums land
             replicated on exactly the partitions of the matching y rows
           - normalize: DVE reciprocal + multiply straight into yT (bf16)
  phase 3: out^T [D, T] partial = W_projT-stationary matmuls over yT.
"""

import os
import numpy as np
import ml_dtypes

import concourse.bass as bass
import concourse.bacc as bacc
import concourse.mybir as mybir
import concourse.tile as tile
from concourse.bass_utils import run_bass_kernel_spmd

# ---- problem constants (hardcoded per harness contract) --------------------
B = 4
T = 2048
D = 1024
H = 16
DH = 64
N_CORES = 8
HPC = H // 2          # heads per core = 8
F = HPC * DH          # 512 = per-core q/k/v feature width

NT = T // 128         # 16 t-tiles
NCC = D // 128        # 8 contraction chunks over D
NQC = T // 512        # 4 q-chunks in attention

F32 = mybir.dt.float32
BF16 = mybir.dt.bfloat16
I16 = mybir.dt.int16

DT = BF16
NP_DT = ml_dtypes.bfloat16

# Schraudolph fast-exp constants: bf16 bits = trunc(s * EXP_A + EXP_B)
# computes ~exp(s/8) with a constant multiplicative bias (cancels in softmax)
EXP_A = float(128.0 * np.log2(np.e) / 8.0)
EXP_B = 16256.0 - 2.75

LAST_EXEC_NS = None
LAST_RESULTS = None


def build_program(debug=False):
    nc = bacc.Bacc()

    xt_d = nc.dram_tensor("x_t", [D, T], DT, kind="ExternalInput")
    wqkv_d = nc.dram_tensor("w_qkv_t", [D, 3 * F], DT, kind="ExternalInput")
    wproj_d = nc.dram_tensor("w_proj_t", [F, D], DT, kind="ExternalInput")
    out_d = nc.dram_tensor("out_p", [D, T], F32, kind="ExternalOutput")
    dbg = {}
    if debug:
        dbg["qkT0"] = nc.dram_tensor("dbg_qkT0", [128, T], DT,
                                     kind="ExternalOutput")
        dbg["qkT4"] = nc.dram_tensor("dbg_qkT4", [128, T], DT,
                                     kind="ExternalOutput")
        dbg["v0"] = nc.dram_tensor("dbg_v0", [128, F], DT,
                                   kind="ExternalOutput")
        dbg["att0"] = nc.dram_tensor("dbg_att0", [128, 1024], DT,
                                     kind="ExternalOutput")
        dbg["d0"] = nc.dram_tensor("dbg_d0", [128, 512], F32,
                                   kind="ExternalOutput")
        dbg["yT0"] = nc.dram_tensor("dbg_yT0", [128, T], DT,
                                    kind="ExternalOutput")

    with tile.TileContext(nc) as tc:
        with (
            tc.tile_pool(name="consts", bufs=1) as consts,
            tc.tile_pool(name="qk_pool", bufs=1) as qk_pool,
            tc.tile_pool(name="v_pool", bufs=1) as v_pool,
            tc.tile_pool(name="y_pool", bufs=1) as y_pool,
            tc.tile_pool(name="wp_pool", bufs=1) as wp_pool,
        ):
            ones64 = consts.tile([128, 64], DT)
            nc.vector.memset(ones64, 1.0)
            # dummy Exp: pulls the ~2.7us ACT_TABLE_LOAD for the exp set
            # into the DMA-bound startup instead of stalling phase 2
            warm = consts.tile([128, 1], F32)
            nc.scalar.activation(warm, ones64[:, 0:1],
                                 mybir.ActivationFunctionType.Exp,
                                 scale=1.0)

            # persistent tensors
            # qkT[f]: f 0..3 -> qT pair f (head 2f rows 0-63, 2f+1 rows
            # 64-127), f 4..7 -> kT pair f-4
            qkT = [qk_pool.tile([128, T], DT, name=f"qkT{f}") for f in range(8)]
            # v[kt]: [128 t, 8 heads * 64]
            v_sb = [v_pool.tile([128, F], DT, name=f"v{t}") for t in range(NT)]
            # yT[hp]: [128 dh (2 heads), T]
            yT = [y_pool.tile([128, T], DT, name=f"yT{hp}") for hp in range(4)]
            # W_proj^T slice tiles [128 dh, D]
            wp_sb = [wp_pool.tile([128, D], DT, name=f"wp{i}") for i in range(4)]

            # ---------------- phase 1: qkv projection ----------------------
            with (
                tc.tile_pool(name="ph1_w", bufs=1) as ph1_w,
                tc.tile_pool(name="ph1_x", bufs=1) as ph1_x,
                tc.tile_pool(name="ph1_psum", bufs=1, space="PSUM") as ph1_p,
            ):
                xs = [ph1_x.tile([128, T], DT, name=f"xs{cc}")
                      for cc in range(NCC)]
                w_sb = [ph1_w.tile([128, 3 * F], DT, name=f"wqkv{cc}")
                        for cc in range(NCC)]
                # first tiles gate the first matmuls: split them into
                # partition-half DMAs so two engines fetch them in parallel
                for half in range(2):
                    p0, p1 = half * 64, (half + 1) * 64
                    nc.sync.dma_start(out=xs[0][p0:p1, :],
                                      in_=xt_d[p0:p1, :])
                    nc.sync.dma_start(out=w_sb[0][p0:p1, :],
                                      in_=wqkv_d[p0:p1, :])
                for cc in range(1, NCC):
                    nc.sync.dma_start(out=xs[cc],
                                      in_=xt_d[cc * 128:(cc + 1) * 128, :])
                    nc.sync.dma_start(out=w_sb[cc],
                                      in_=wqkv_d[cc * 128:(cc + 1) * 128, :])
                for i in range(4):
                    nc.sync.dma_start(out=wp_sb[i],
                                      in_=wproj_d[i * 128:(i + 1) * 128, :])

                # q/k: out [f 128, t 512]; w stationary amortized over 2 tc
                for f in range(8):
                    for tcb in range(2):
                        ps = [ph1_p.tile([128, 512], F32, name="ps_qk",
                                         tag=f"qk{i}", bufs=2)
                              for i in range(2)]
                        for cc in range(NCC):
                            for i in range(2):
                                tci = tcb * 2 + i
                                nc.tensor.matmul(
                                    ps[i],
                                    lhsT=w_sb[cc][:, f * 128:(f + 1) * 128],
                                    rhs=xs[cc][:, tci * 512:(tci + 1) * 512],
                                    start=(cc == 0), stop=(cc == NCC - 1))
                        for i in range(2):
                            tci = tcb * 2 + i
                            dst = qkT[f][:, tci * 512:(tci + 1) * 512]
                            if i == 0:
                                nc.scalar.activation(
                                    dst, ps[i],
                                    mybir.ActivationFunctionType.Copy)
                            else:
                                nc.vector.tensor_copy(dst, ps[i])

                # v: out [t 128, f 512], xT slice stationary
                for kt in range(NT):
                    ps_v = ph1_p.tile([128, F], F32, name="ps_v",
                                      tag="ps_v", bufs=2)
                    for cc in range(NCC):
                        nc.tensor.matmul(
                            ps_v,
                            lhsT=xs[cc][:, kt * 128:(kt + 1) * 128],
                            rhs=w_sb[cc][:, 2 * F:3 * F],
                            start=(cc == 0), stop=(cc == NCC - 1))
                    if kt % 2 == 0:
                        nc.vector.tensor_copy(v_sb[kt], ps_v)
                    else:
                        nc.scalar.activation(
                            v_sb[kt], ps_v, mybir.ActivationFunctionType.Copy)

            if debug:
                nc.sync.dma_start(out=dbg["qkT0"][:, :], in_=qkT[0])
                nc.sync.dma_start(out=dbg["qkT4"][:, :], in_=qkT[4])
                nc.sync.dma_start(out=dbg["v0"][:, :], in_=v_sb[0])

            # ---------------- phase 2: attention ---------------------------
            # pair-sequential, qc-paired: per (head-pair hp) kt loop covers
            # two 512-q chunks so every score/AV/den LDWEIGHTS feeds 2 MMs.
            # Per-head [128,1024] score tiles (2 rotating slots); head 0 of
            # each pair -> ScalarE native Exp, head 1 -> DVE fast-exp.
            with (
                tc.tile_pool(name="ph2_s", bufs=1) as ph2_s,
                tc.tile_pool(name="ph2_psum", bufs=1, space="PSUM") as ph2_p,
            ):
                for qcp in range(2):
                    q0 = qcp * 1024
                    for hp in range(4):
                        h0, h1 = 2 * hp, 2 * hp + 1
                        kTp, qTp = qkT[4 + hp], qkT[hp]
                        ps_y = [ph2_p.tile([128, 512], F32, name="ps_y",
                                           tag=f"y{qi}", bufs=1)
                                for qi in range(2)]
                        ps_dn = [ph2_p.tile([128, 512], F32, name="ps_d",
                                            tag=f"d{qi}", bufs=1)
                                 for qi in range(2)]
                        att = [None, None]
                        # software pipeline: scores(kt) + AV/den(kt-1)
                        for kt in range(NT + 1):
                            if kt < NT:
                                sA = ph2_p.tile([128, 1024], F32, name="ps_s",
                                                tag="s", bufs=2)
                                sB = ph2_p.tile([128, 1024], F32, name="ps_s",
                                                tag="s", bufs=2)
                                # interleave A/B so row-halves overlap and
                                # each LDW (kT slice) feeds 2 q-chunk MMs
                                for qi in range(2):
                                    qs = q0 + qi * 512
                                    nc.tensor.matmul(
                                        sA[:, qi * 512:(qi + 1) * 512],
                                        lhsT=kTp[0:64,
                                                 kt * 128:(kt + 1) * 128],
                                        rhs=qTp[0:64, qs:qs + 512],
                                        start=True, stop=True)
                                    nc.tensor.matmul(
                                        sB[:, qi * 512:(qi + 1) * 512],
                                        lhsT=kTp[64:128,
                                                 kt * 128:(kt + 1) * 128],
                                        rhs=qTp[64:128, qs:qs + 512],
                                        start=True, stop=True)
                                aA = ph2_s.tile([128, 1024], DT, name="att",
                                                tag="att", bufs=4)
                                aB = ph2_s.tile([128, 1024], DT, name="att",
                                                tag="att", bufs=4)
                                nc.scalar.activation(
                                    aA, sA, mybir.ActivationFunctionType.Exp,
                                    scale=1.0 / 8.0)
                                nc.vector.tensor_scalar(
                                    aB.bitcast(I16), sB, EXP_A, EXP_B,
                                    mybir.AluOpType.mult,
                                    mybir.AluOpType.add)
                                att_new = [aA, aB]
                            if kt > 0:
                                ko = kt - 1
                                a0, a1 = att
                                st, sp = (ko == 0), (ko == NT - 1)
                                # AV cross-packed: col halves 0/1 concurrent
                                nc.tensor.matmul(
                                    ps_y[0][0:64, :],
                                    lhsT=v_sb[ko][:, h0 * 64:(h0 + 1) * 64],
                                    rhs=a0[:, 0:512], start=st, stop=sp)
                                nc.tensor.matmul(
                                    ps_y[1][64:128, :],
                                    lhsT=v_sb[ko][:, h1 * 64:(h1 + 1) * 64],
                                    rhs=a1[:, 512:1024], start=st, stop=sp)
                                nc.tensor.matmul(
                                    ps_y[1][0:64, :],
                                    lhsT=v_sb[ko][:, h0 * 64:(h0 + 1) * 64],
                                    rhs=a0[:, 512:1024], start=st, stop=sp)
                                nc.tensor.matmul(
                                    ps_y[0][64:128, :],
                                    lhsT=v_sb[ko][:, h1 * 64:(h1 + 1) * 64],
                                    rhs=a1[:, 0:512], start=st, stop=sp)
                                # den cross-packed
                                nc.tensor.matmul(
                                    ps_dn[0][0:64, :], lhsT=ones64,
                                    rhs=a0[:, 0:512], start=st, stop=sp)
                                nc.tensor.matmul(
                                    ps_dn[1][64:128, :], lhsT=ones64,
                                    rhs=a1[:, 512:1024], start=st, stop=sp)
                                nc.tensor.matmul(
                                    ps_dn[1][0:64, :], lhsT=ones64,
                                    rhs=a0[:, 512:1024], start=st, stop=sp)
                                nc.tensor.matmul(
                                    ps_dn[0][64:128, :], lhsT=ones64,
                                    rhs=a1[:, 0:512], start=st, stop=sp)
                            if kt < NT:
                                att = att_new
                        # normalize into yT
                        for qi in range(2):
                            r = ph2_s.tile([128, 512], F32, name="r",
                                           tag="r", bufs=2)
                            nc.vector.reciprocal_approx_fast(r, ps_dn[qi])
                            nc.vector.tensor_mul(
                                yT[hp][:, q0 + qi * 512:q0 + (qi + 1) * 512],
                                ps_y[qi], r)

            # ---------------- phase 3: output projection -------------------
            # out^T [o 128, t]: W_projT slice stationary, yT moving
            with (
                tc.tile_pool(name="ph3_s", bufs=1) as ph3_s,
                tc.tile_pool(name="ph3_psum", bufs=1, space="PSUM") as ph3_p,
            ):
                for oc in range(8):
                    ps_o = [ph3_p.tile([128, 512], F32, name="ps_o",
                                       tag=f"o{i}", bufs=2)
                            for i in range(4)]
                    for hp in range(4):
                        for tcq in range(4):
                            nc.tensor.matmul(
                                ps_o[tcq],
                                lhsT=wp_sb[hp][:, oc * 128:(oc + 1) * 128],
                                rhs=yT[hp][:, tcq * 512:(tcq + 1) * 512],
                                start=(hp == 0), stop=(hp == 3))
                    o_sb = ph3_s.tile([128, T], F32, name="o_sb", tag="o_sb",
                                      bufs=2)
                    for tcq in range(4):
                        dst = o_sb[:, tcq * 512:(tcq + 1) * 512]
                        if tcq % 2 == 0:
                            nc.vector.tensor_copy(dst, ps_o[tcq])
                        else:
                            nc.scalar.activation(
                                dst, ps_o[tcq],
                                mybir.ActivationFunctionType.Copy)
                    nc.sync.dma_start(out=out_d[oc * 128:(oc + 1) * 128, :],
                                      in_=o_sb)
    return nc


_NC_CACHE = None


def _get_program():
    global _NC_CACHE
    if _NC_CACHE is None:
        nc = build_program()
        if not nc.is_finalized():
            nc.finalize()
        _NC_CACHE = nc
    return _NC_CACHE


def make_in_maps(x, W_qkv, W_proj):
    """Shard full inputs into per-core input maps (host-side layout prep)."""
    Wq, Wk, Wv = W_qkv[0:D], W_qkv[D:2 * D], W_qkv[2 * D:3 * D]
    maps = []
    wq_g, wp_g = {}, {}
    for g in range(2):
        rows = slice(g * F, (g + 1) * F)
        wq_g[g] = np.ascontiguousarray(
            np.concatenate([Wq[rows].T, Wk[rows].T, Wv[rows].T], axis=1)
        ).astype(NP_DT)
        wp_g[g] = np.ascontiguousarray(W_proj[:, rows].T).astype(NP_DT)
    xt_b = [np.ascontiguousarray(x[b].T).astype(NP_DT) for b in range(B)]
    for core in range(N_CORES):
        b, g = core // 2, core % 2
        maps.append({
            "x_t": xt_b[b],
            "w_qkv_t": wq_g[g],
            "w_proj_t": wp_g[g],
        })
    return maps


def kernel(x, W_qkv, W_proj):
    global LAST_EXEC_NS, LAST_RESULTS
    x = np.asarray(x, dtype=np.float32)
    W_qkv = np.asarray(W_qkv, dtype=np.float32)
    W_proj = np.asarray(W_proj, dtype=np.float32)

    nc = _get_program()
    in_maps = make_in_maps(x, W_qkv, W_proj)
    trace = bool(int(os.environ.get("BASS_KERNEL_TRACE", "0")))
    res = run_bass_kernel_spmd(nc, in_maps, list(range(N_CORES)), trace=trace)
    LAST_EXEC_NS = res.exec_time_ns
    LAST_RESULTS = res
    out = np.stack([
        np.ascontiguousarray(
            (np.asarray(res.results[2 * b]["out_p"], dtype=np.float32)
             + np.asarray(res.results[2 * b + 1]["out_p"], dtype=np.float32)).T)
        for b in range(B)
    ])
    return out


# revision 24
# speedup vs baseline: 1.1770x; 1.1770x over previous
"""Bass/Tile kernel for bidirectional multi-head self-attention on 8 trn2 cores.

Problem: x[4, 2048, 1024], W_qkv[3072, 1024], W_proj[1024, 1024], H=16 heads,
Dh=64.  out = proj(softmax(q k^T / sqrt(Dh)) v).

Sharding: core c = (batch b = c//2, head-group g = c%2).  Each core computes
attention for 8 heads of one batch and a full-T partial output projection
(contraction over its 512 C_in columns); host sums the pair partials and
stacks batches.  x is transposed on the host so no PE transposes are needed.

Per-core pipeline (all matmuls bf16 in / fp32 psum accumulate):
  phase 1: qT/kT [dh,T] from stationary-W matmuls over xT; v [T,dh] tiles.
  phase 2: per head-quad, software-pipelined over kt:
           - scores row-packed: 2 heads concurrently on array row-halves
             (K=64 contraction each) into one [128,1024] psum pair tile
           - exp split between ScalarE (native Exp) and DVE (Schraudolph
             int16-bitcast fast exp; its constant bias cancels in softmax)
           - AV col-packed: 2 heads concurrently on array col-halves
           - denominators via ones[128,64] lhsT matmuls: row sums land
             replicated on exactly the partitions of the matching y rows
           - normalize: DVE reciprocal + multiply straight into yT (bf16)
  phase 3: out^T [D, T] partial = W_projT-stationary matmuls over yT.
"""

import os
import numpy as np
import ml_dtypes

import concourse.bass as bass
import concourse.bacc as bacc
import concourse.mybir as mybir
import concourse.tile as tile
from concourse.bass_utils import run_bass_kernel_spmd

# ---- problem constants (hardcoded per harness contract) --------------------
B = 4
T = 2048
D = 1024
H = 16
DH = 64
N_CORES = 8
HPC = H // 2          # heads per core = 8
F = HPC * DH          # 512 = per-core q/k/v feature width

NT = T // 128         # 16 t-tiles
NCC = D // 128        # 8 contraction chunks over D
NQC = T // 512        # 4 q-chunks in attention

F32 = mybir.dt.float32
BF16 = mybir.dt.bfloat16
I16 = mybir.dt.int16

DT = BF16
NP_DT = ml_dtypes.bfloat16

# Schraudolph fast-exp constants: bf16 bits = trunc(s * EXP_A + EXP_B)
# computes ~exp(s/8) with a constant multiplicative bias (cancels in softmax)
EXP_A = float(128.0 * np.log2(np.e) / 8.0)
EXP_B = 16256.0 - 2.75

LAST_EXEC_NS = None
LAST_RESULTS = None


def build_program(debug=False):
    nc = bacc.Bacc()

    xt_d = nc.dram_tensor("x_t", [D, T], DT, kind="ExternalInput")
    wqkv_d = nc.dram_tensor("w_qkv_t", [D, 3 * F], DT, kind="ExternalInput")
    wproj_d = nc.dram_tensor("w_proj_t", [F, D], DT, kind="ExternalInput")
    out_d = nc.dram_tensor("out_p", [D, T], DT, kind="ExternalOutput")
    dbg = {}
    if debug:
        dbg["qkT0"] = nc.dram_tensor("dbg_qkT0", [128, T], DT,
                                     kind="ExternalOutput")
        dbg["qkT4"] = nc.dram_tensor("dbg_qkT4", [128, T], DT,
                                     kind="ExternalOutput")
        dbg["v0"] = nc.dram_tensor("dbg_v0", [128, F], DT,
                                   kind="ExternalOutput")
        dbg["att0"] = nc.dram_tensor("dbg_att0", [128, 1024], DT,
                                     kind="ExternalOutput")
        dbg["d0"] = nc.dram_tensor("dbg_d0", [128, 512], F32,
                                   kind="ExternalOutput")
        dbg["yT0"] = nc.dram_tensor("dbg_yT0", [128, T], DT,
                                    kind="ExternalOutput")

    with tile.TileContext(nc) as tc:
        with (
            tc.tile_pool(name="consts", bufs=1) as consts,
            tc.tile_pool(name="qk_pool", bufs=1) as qk_pool,
            tc.tile_pool(name="v_pool", bufs=1) as v_pool,
            tc.tile_pool(name="y_pool", bufs=1) as y_pool,
            tc.tile_pool(name="wp_pool", bufs=1) as wp_pool,
        ):
            ones64 = consts.tile([128, 64], DT)
            nc.vector.memset(ones64, 1.0)
            # dummy Exp: pulls the ~2.7us ACT_TABLE_LOAD for the exp set
            # into the DMA-bound startup instead of stalling phase 2
            warm = consts.tile([128, 1], F32)
            nc.scalar.activation(warm, ones64[:, 0:1],
                                 mybir.ActivationFunctionType.Exp,
                                 scale=1.0)

            # persistent tensors
            # qkT[f]: f 0..3 -> qT pair f (head 2f rows 0-63, 2f+1 rows
            # 64-127), f 4..7 -> kT pair f-4
            qkT = [qk_pool.tile([128, T], DT, name=f"qkT{f}") for f in range(8)]
            # v[kt]: [128 t, 8 heads * 64]
            v_sb = [v_pool.tile([128, F], DT, name=f"v{t}") for t in range(NT)]
            # yT[hp]: [128 dh (2 heads), T]
            yT = [y_pool.tile([128, T], DT, name=f"yT{hp}") for hp in range(4)]
            # W_proj^T slice tiles [128 dh, D]
            wp_sb = [wp_pool.tile([128, D], DT, name=f"wp{i}") for i in range(4)]

            # ---------------- phase 1: qkv projection ----------------------
            with (
                tc.tile_pool(name="ph1_w", bufs=1) as ph1_w,
                tc.tile_pool(name="ph1_x", bufs=1) as ph1_x,
                tc.tile_pool(name="ph1_psum", bufs=1, space="PSUM") as ph1_p,
            ):
                xs = [ph1_x.tile([128, T], DT, name=f"xs{cc}")
                      for cc in range(NCC)]
                w_sb = [ph1_w.tile([128, 3 * F], DT, name=f"wqkv{cc}")
                        for cc in range(NCC)]
                # first tiles gate the first matmuls: split them so
                # several engines fetch them in parallel
                for q in range(4):
                    p0, p1 = q * 32, (q + 1) * 32
                    nc.sync.dma_start(out=xs[0][p0:p1, :],
                                      in_=xt_d[p0:p1, :])
                    nc.sync.dma_start(out=w_sb[0][p0:p1, :],
                                      in_=wqkv_d[p0:p1, :])
                for half in range(2):
                    p0, p1 = half * 64, (half + 1) * 64
                    nc.sync.dma_start(out=xs[1][p0:p1, :],
                                      in_=xt_d[128 + p0:128 + p1, :])
                    nc.sync.dma_start(out=w_sb[1][p0:p1, :],
                                      in_=wqkv_d[128 + p0:128 + p1, :])
                for cc in range(2, NCC):
                    nc.sync.dma_start(out=xs[cc],
                                      in_=xt_d[cc * 128:(cc + 1) * 128, :])
                    nc.sync.dma_start(out=w_sb[cc],
                                      in_=wqkv_d[cc * 128:(cc + 1) * 128, :])
                for i in range(4):
                    nc.sync.dma_start(out=wp_sb[i],
                                      in_=wproj_d[i * 128:(i + 1) * 128, :])

                # q/k: out [f 128, t 512]; w stationary amortized over 2 tc
                for f in range(8):
                    for tcb in range(2):
                        ps = [ph1_p.tile([128, 512], F32, name="ps_qk",
                                         tag=f"qk{i}", bufs=2)
                              for i in range(2)]
                        for cc in range(NCC):
                            for i in range(2):
                                tci = tcb * 2 + i
                                nc.tensor.matmul(
                                    ps[i],
                                    lhsT=w_sb[cc][:, f * 128:(f + 1) * 128],
                                    rhs=xs[cc][:, tci * 512:(tci + 1) * 512],
                                    start=(cc == 0), stop=(cc == NCC - 1))
                        for i in range(2):
                            tci = tcb * 2 + i
                            dst = qkT[f][:, tci * 512:(tci + 1) * 512]
                            if i == 0:
                                nc.scalar.activation(
                                    dst, ps[i],
                                    mybir.ActivationFunctionType.Copy)
                            else:
                                nc.vector.tensor_copy(dst, ps[i])

                # v: out [t 128, f 512], xT slice stationary
                for kt in range(NT):
                    ps_v = ph1_p.tile([128, F], F32, name="ps_v",
                                      tag="ps_v", bufs=2)
                    for cc in range(NCC):
                        nc.tensor.matmul(
                            ps_v,
                            lhsT=xs[cc][:, kt * 128:(kt + 1) * 128],
                            rhs=w_sb[cc][:, 2 * F:3 * F],
                            start=(cc == 0), stop=(cc == NCC - 1))
                    if kt % 2 == 0:
                        nc.vector.tensor_copy(v_sb[kt], ps_v)
                    else:
                        nc.scalar.activation(
                            v_sb[kt], ps_v, mybir.ActivationFunctionType.Copy)

            if debug:
                nc.sync.dma_start(out=dbg["qkT0"][:, :], in_=qkT[0])
                nc.sync.dma_start(out=dbg["qkT4"][:, :], in_=qkT[4])
                nc.sync.dma_start(out=dbg["v0"][:, :], in_=v_sb[0])

            # ---------------- phase 2: attention ---------------------------
            # pair-sequential, qc-paired: per (head-pair hp) kt loop covers
            # two 512-q chunks so every score/AV/den LDWEIGHTS feeds 2 MMs.
            # Per-head [128,1024] score tiles (2 rotating slots); head 0 of
            # each pair -> ScalarE native Exp, head 1 -> DVE fast-exp.
            with (
                tc.tile_pool(name="ph2_s", bufs=1) as ph2_s,
                tc.tile_pool(name="ph2_psum", bufs=1, space="PSUM") as ph2_p,
            ):
                for qcp in range(2):
                    q0 = qcp * 1024
                    for hp in range(4):
                        h0, h1 = 2 * hp, 2 * hp + 1
                        kTp, qTp = qkT[4 + hp], qkT[hp]
                        ps_y = [ph2_p.tile([128, 512], F32, name="ps_y",
                                           tag=f"y{qi}", bufs=1)
                                for qi in range(2)]
                        ps_dn = [ph2_p.tile([128, 512], F32, name="ps_d",
                                            tag=f"d{qi}", bufs=1)
                                 for qi in range(2)]
                        att = [None, None]
                        # software pipeline: scores(kt) + AV/den(kt-1)
                        for kt in range(NT + 1):
                            if kt < NT:
                                sA = ph2_p.tile([128, 1024], F32, name="ps_s",
                                                tag="s", bufs=2)
                                sB = ph2_p.tile([128, 1024], F32, name="ps_s",
                                                tag="s", bufs=2)
                                # interleave A/B so row-halves overlap and
                                # each LDW (kT slice) feeds 2 q-chunk MMs
                                for qi in range(2):
                                    qs = q0 + qi * 512
                                    nc.tensor.matmul(
                                        sA[:, qi * 512:(qi + 1) * 512],
                                        lhsT=kTp[0:64,
                                                 kt * 128:(kt + 1) * 128],
                                        rhs=qTp[0:64, qs:qs + 512],
                                        start=True, stop=True)
                                    nc.tensor.matmul(
                                        sB[:, qi * 512:(qi + 1) * 512],
                                        lhsT=kTp[64:128,
                                                 kt * 128:(kt + 1) * 128],
                                        rhs=qTp[64:128, qs:qs + 512],
                                        start=True, stop=True)
                                aA = ph2_s.tile([128, 1024], DT, name="att",
                                                tag="att", bufs=4)
                                aB = ph2_s.tile([128, 1024], DT, name="att",
                                                tag="att", bufs=4)
                                nc.scalar.activation(
                                    aA, sA, mybir.ActivationFunctionType.Exp,
                                    scale=1.0 / 8.0)
                                nc.vector.tensor_scalar(
                                    aB.bitcast(I16), sB, EXP_A, EXP_B,
                                    mybir.AluOpType.mult,
                                    mybir.AluOpType.add)
                                att_new = [aA, aB]
                            if kt > 0:
                                ko = kt - 1
                                a0, a1 = att
                                st, sp = (ko == 0), (ko == NT - 1)
                                # AV cross-packed: col halves 0/1 concurrent
                                nc.tensor.matmul(
                                    ps_y[0][0:64, :],
                                    lhsT=v_sb[ko][:, h0 * 64:(h0 + 1) * 64],
                                    rhs=a0[:, 0:512], start=st, stop=sp)
                                nc.tensor.matmul(
                                    ps_y[1][64:128, :],
                                    lhsT=v_sb[ko][:, h1 * 64:(h1 + 1) * 64],
                                    rhs=a1[:, 512:1024], start=st, stop=sp)
                                nc.tensor.matmul(
                                    ps_y[1][0:64, :],
                                    lhsT=v_sb[ko][:, h0 * 64:(h0 + 1) * 64],
                                    rhs=a0[:, 512:1024], start=st, stop=sp)
                                nc.tensor.matmul(
                                    ps_y[0][64:128, :],
                                    lhsT=v_sb[ko][:, h1 * 64:(h1 + 1) * 64],
                                    rhs=a1[:, 0:512], start=st, stop=sp)
                                # den cross-packed
                                nc.tensor.matmul(
                                    ps_dn[0][0:64, :], lhsT=ones64,
                                    rhs=a0[:, 0:512], start=st, stop=sp)
                                nc.tensor.matmul(
                                    ps_dn[1][64:128, :], lhsT=ones64,
                                    rhs=a1[:, 512:1024], start=st, stop=sp)
                                nc.tensor.matmul(
                                    ps_dn[1][0:64, :], lhsT=ones64,
                                    rhs=a0[:, 512:1024], start=st, stop=sp)
                                nc.tensor.matmul(
                                    ps_dn[0][64:128, :], lhsT=ones64,
                                    rhs=a1[:, 0:512], start=st, stop=sp)
                            if kt < NT:
                                att = att_new
                        # normalize into yT
                        for qi in range(2):
                            r = ph2_s.tile([128, 512], F32, name="r",
                                           tag="r", bufs=2)
                            nc.vector.reciprocal_approx_fast(r, ps_dn[qi])
                            nc.vector.tensor_mul(
                                yT[hp][:, q0 + qi * 512:q0 + (qi + 1) * 512],
                                ps_y[qi], r)

            # ---------------- phase 3: output projection -------------------
            # out^T [o 128, t]: W_projT slice stationary, yT moving
            with (
                tc.tile_pool(name="ph3_s", bufs=1) as ph3_s,
                tc.tile_pool(name="ph3_psum", bufs=1, space="PSUM") as ph3_p,
            ):
                for oc in range(8):
                    ps_o = [ph3_p.tile([128, 512], F32, name="ps_o",
                                       tag=f"o{i}", bufs=2)
                            for i in range(4)]
                    for hp in range(4):
                        for tcq in range(4):
                            nc.tensor.matmul(
                                ps_o[tcq],
                                lhsT=wp_sb[hp][:, oc * 128:(oc + 1) * 128],
                                rhs=yT[hp][:, tcq * 512:(tcq + 1) * 512],
                                start=(hp == 0), stop=(hp == 3))
                    o_sb = ph3_s.tile([128, T], DT, name="o_sb", tag="o_sb",
                                      bufs=2)
                    for tcq in range(4):
                        dst = o_sb[:, tcq * 512:(tcq + 1) * 512]
                        if tcq % 2 == 0:
                            nc.vector.tensor_copy(dst, ps_o[tcq])
                        else:
                            nc.scalar.activation(
                                dst, ps_o[tcq],
                                mybir.ActivationFunctionType.Copy)
                    nc.sync.dma_start(out=out_d[oc * 128:(oc + 1) * 128, :],
                                      in_=o_sb)
    return nc


_NC_CACHE = None


def _get_program():
    global _NC_CACHE
    if _NC_CACHE is None:
        nc = build_program()
        if not nc.is_finalized():
            nc.finalize()
        _NC_CACHE = nc
    return _NC_CACHE


def make_in_maps(x, W_qkv, W_proj):
    """Shard full inputs into per-core input maps (host-side layout prep)."""
    Wq, Wk, Wv = W_qkv[0:D], W_qkv[D:2 * D], W_qkv[2 * D:3 * D]
    maps = []
    wq_g, wp_g = {}, {}
    for g in range(2):
        rows = slice(g * F, (g + 1) * F)
        wq_g[g] = np.ascontiguousarray(
            np.concatenate([Wq[rows].T, Wk[rows].T, Wv[rows].T], axis=1)
        ).astype(NP_DT)
        wp_g[g] = np.ascontiguousarray(W_proj[:, rows].T).astype(NP_DT)
    xt_b = [np.ascontiguousarray(x[b].T).astype(NP_DT) for b in range(B)]
    for core in range(N_CORES):
        b, g = core // 2, core % 2
        maps.append({
            "x_t": xt_b[b],
            "w_qkv_t": wq_g[g],
            "w_proj_t": wp_g[g],
        })
    return maps


def kernel(x, W_qkv, W_proj):
    global LAST_EXEC_NS, LAST_RESULTS
    x = np.asarray(x, dtype=np.float32)
    W_qkv = np.asarray(W_qkv, dtype=np.float32)
    W_proj = np.asarray(W_proj, dtype=np.float32)

    nc = _get_program()
    in_maps = make_in_maps(x, W_qkv, W_proj)
    trace = bool(int(os.environ.get("BASS_KERNEL_TRACE", "0")))
    res = run_bass_kernel_spmd(nc, in_maps, list(range(N_CORES)), trace=trace)
    LAST_EXEC_NS = res.exec_time_ns
    LAST_RESULTS = res
    out = np.stack([
        np.ascontiguousarray(
            (np.asarray(res.results[2 * b]["out_p"], dtype=np.float32)
             + np.asarray(res.results[2 * b + 1]["out_p"], dtype=np.float32)).T)
        for b in range(B)
    ])
    return out
